# revision 7
# baseline (speedup 1.0000x reference)
"""GraphMAE (GAT encoder/decoder) forward on 8 Trainium2 NeuronCores.

Strategy (graph/data parallel, per sharding hint):
  - Nodes are partitioned contiguously across the 8 cores (6250 each).
  - Per-layer node-wise GEMMs are computed on the owning core; weights are
    replicated. After each dense stage, the per-node rows [h | el] are
    AllGathered into a replicated DRAM "gather table".
  - Edges are partitioned by destination owner. Each core processes its
    ~100k edges (sorted by dst, in blocks of 128 dsts, tiles of 128 edges):
      * indirect-DMA gather of [h|el] rows by src index from the table
      * per-edge er via a 0/1 selector matmul (seg -> dst within block)
      * p = exp(leaky_relu(el+er))  (no per-dst max subtraction: score
        ranges are small enough that raw exp stays in f32 range, and the
        softmax normalization divides it out exactly)
      * segment-sum of [p*h | p] via selector matmul accumulated in PSUM
      * per-dst division (+1e-9) reproduces the reference softmax.
  - The SCE loss is computed blockwise on the owning core and reduced on
    the host (sum of per-core partials / n_mask).

The kernel program is SPMD: one Bass module, per-core input maps.
"""
import os
import sys
import math

sys.path.insert(0, '/opt/trn_rl_repo')

import numpy as np
from contextlib import ExitStack

from concourse import bass, mybir, tile, bacc
from concourse.bass_utils import run_bass_kernel_spmd
from concourse.masks import make_identity

# ---------------- problem constants (hardcoded per harness contract) ----
N = 50000
E = 800000
D = 256
H = 4
DH = 64
NEG_SLOPE = 0.2
NUM_MASK = 25000

NCORES = 8
NL = N // NCORES          # 6250 nodes per core
P = 128
NBLK = math.ceil(NL / P)  # 49 blocks (last has 106 dsts)
LASTB = NL - (NBLK - 1) * P  # 106

ROW_E = D + H             # encoder table row: [h(256) | el(4)]
ROW_D = D + 2             # decoder table row: [h(256) | el(1) | pad] (even width for fp32r)
TROWS = N + P             # table rows incl. pad row at index N
PADROW = N

MM_MODE = os.environ.get("KMM", "f32r")   # f32r | f32
N_EDGE_LAYERS = int(os.environ.get("KLAYERS", "3"))

F32 = mybir.dt.float32
F32R = mybir.dt.float32r
I32 = mybir.dt.int32
MMDT = {"f32r": F32R, "f32": F32}[MM_MODE]

AF = mybir.ActivationFunctionType
OP = mybir.AluOpType


# ======================= host-side sharding =============================

def _shard(inputs):
    """Build per-core index/mask structures and the shared tile schedule."""
    es = np.asarray(inputs["edge_src"]).astype(np.int64)
    ed = np.asarray(inputs["edge_dst"]).astype(np.int64)
    x = np.asarray(inputs["x"], dtype=np.float32)
    token = np.asarray(inputs["token_nodes"]).astype(np.int64)
    noise = np.asarray(inputs["noise_nodes"]).astype(np.int64)
    noise_src = np.asarray(inputs["noise_src"]).astype(np.int64)
    maskn = np.asarray(inputs["mask_nodes"]).astype(np.int64)

    owner = ed // NL
    order = np.argsort(ed, kind="stable")
    es_s, ed_s = es[order], ed[order]
    # per-core slices (dst-sorted)
    core_edges = []
    bounds = np.searchsorted(ed_s, np.arange(NCORES + 1) * NL)
    for c in range(NCORES):
        lo, hi = bounds[c], bounds[c + 1]
        core_edges.append((es_s[lo:hi], ed_s[lo:hi] - c * NL))

    # per (core, block) edge counts -> shared tile schedule
    tiles_b = np.zeros(NBLK, dtype=np.int64)
    blk_slices = []
    for c in range(NCORES):
        e_src, e_dstl = core_edges[c]
        bb = np.searchsorted(e_dstl, np.arange(NBLK + 1) * P)
        blk_slices.append(bb)
        cnt = np.diff(bb)
        tiles_b = np.maximum(tiles_b, (cnt + P - 1) // P)
    tiles_b = np.maximum(tiles_b, 1)
    T = int(tiles_b.sum())

    # per-core gsrc/gseg in [128, T] tile-column layout
    gsrc = np.full((NCORES, P, T), PADROW, dtype=np.int32)
    gseg = np.zeros((NCORES, P, T), dtype=np.float32)
    col0 = np.concatenate([[0], np.cumsum(tiles_b)])
    for c in range(NCORES):
        e_src, e_dstl = core_edges[c]
        bb = blk_slices[c]
        for b in range(NBLK):
            lo, hi = bb[b], bb[b + 1]
            nb = hi - lo
            tb = int(tiles_b[b])
            src_pad = np.full(tb * P, PADROW, dtype=np.int32)
            seg_pad = np.zeros(tb * P, dtype=np.float32)
            src_pad[:nb] = e_src[lo:hi]
            seg_pad[:nb] = (e_dstl[lo:hi] - b * P).astype(np.float32)
            gsrc[c, :, col0[b]:col0[b] + tb] = src_pad.reshape(tb, P).T
            gseg[c, :, col0[b]:col0[b] + tb] = seg_pad.reshape(tb, P).T

    # masks, per core, [128, NBLK] block-column layout (rows padded to 6272)
    NPAD = NBLK * P
    tokflag = np.zeros(N, np.float32); tokflag[token] = 1.0
    maskflag = np.zeros(N, np.float32); maskflag[maskn] = 1.0

    def cols(v, c):
        w = np.zeros(NPAD, v.dtype)
        w[:NL] = v[c * NL:(c + 1) * NL]
        return w.reshape(NBLK, P).T.copy()

    # xg table: local x rows + this core's noise replacement rows
    per_core = []
    NXG = NL + 512  # fixed shape across cores
    for c in range(NCORES):
        nsel = (noise >= c * NL) & (noise < (c + 1) * NL)
        nl_local = noise[nsel] - c * NL
        nsrc_local = noise_src[nsel]
        assert nl_local.size <= 512
        xg = np.zeros((NXG, D), np.float32)
        xg[:NL] = x[c * NL:(c + 1) * NL]
        xg[NL:NL + nl_local.size] = x[nsrc_local]
        pi = np.arange(NPAD, dtype=np.int32)
        pi[NL:] = 0
        pi[nl_local] = NL + np.arange(nl_local.size, dtype=np.int32)
        per_core.append({
            "xg": xg,
            "pi": pi.reshape(NBLK, P).T.copy().astype(np.int32),
            "tokflag": cols(tokflag, c),
            "tokkeep": 1.0 - cols(tokflag, c),
            "maskkeep": 1.0 - cols(maskflag, c),
            "lossflag": cols(maskflag, c),
            "gsrc": np.ascontiguousarray(gsrc[c]),
            "gseg": np.ascontiguousarray(gseg[c]),
        })
    return per_core, [int(t) for t in tiles_b], NXG


# ======================= device program =================================

def _build(tiles_b, NXG):
    nc = bacc.Bacc("TRN2", debug=False, num_devices=NCORES)
    T = sum(tiles_b)

    def din(name, shape, dt=F32):
        return nc.dram_tensor(name, shape, dt, kind="ExternalInput").ap()

    xg_d = din("xg", [NXG, D])
    pi_d = din("pi", [P, NBLK], I32)
    tokflag_d = din("tokflag", [P, NBLK])
    tokkeep_d = din("tokkeep", [P, NBLK])
    maskkeep_d = din("maskkeep", [P, NBLK])
    lossflag_d = din("lossflag", [P, NBLK])
    gsrc_d = din("gsrc", [P, T], I32)
    gseg_d = din("gseg", [P, T])
    W0_d = din("W0", [D, D]); W1_d = din("W1", [D, D])
    We_d = din("We2d", [D, D]); Wd_d = din("Wd", [D, D])
    al0_d = din("al0", [1, D]); ar0_d = din("ar0", [1, D])
    al1_d = din("al1", [1, D]); ar1_d = din("ar1", [1, D])
    ald_d = din("ald", [1, D]); ard_d = din("ard", [1, D])
    mtok_d = din("mtok", [1, D])

    out_enc = nc.dram_tensor("enc_rep", [NL, D], F32, kind="ExternalOutput").ap()
    out_loss = nc.dram_tensor("loss_part", [1, 1], F32, kind="ExternalOutput").ap()

    with tile.TileContext(nc) as tc, ExitStack() as ctx:
        # pools
        const = ctx.enter_context(tc.tile_pool(name="const", bufs=1))
        meta = ctx.enter_context(tc.tile_pool(name="meta", bufs=1))
        sbg = ctx.enter_context(tc.tile_pool(name="sbg", bufs=6))      # gathered rows
        sbw = ctx.enter_context(tc.tile_pool(name="sbw", bufs=4))      # work tiles
        sbs = ctx.enter_context(tc.tile_pool(name="sbs", bufs=6))      # small tiles
        res = ctx.enter_context(tc.tile_pool(name="res", bufs=1))      # layer residents
        ps_acc = ctx.enter_context(tc.tile_pool(name="ps_acc", bufs=2, space="PSUM"))
        ps_er = ctx.enter_context(tc.tile_pool(name="ps_er", bufs=2, space="PSUM"))
        ps_big = ctx.enter_context(tc.tile_pool(name="ps_big", bufs=2, space="PSUM"))
        dram = ctx.enter_context(tc.tile_pool(name="dram", bufs=1, space="DRAM"))

        # ---------------- constants / weights -------------------------
        ident = const.tile([P, P], F32)
        make_identity(nc, ident[:])
        ident_mm = const.tile([P, P], MMDT)
        nc.vector.tensor_copy(out=ident_mm[:], in_=ident[:])
        iota = const.tile([P, P], F32)
        nc.gpsimd.iota(iota[:], pattern=[[1, P]], base=0, channel_multiplier=0,
                       allow_small_or_imprecise_dtypes=True)
        ones_col = const.tile([P, 1], F32)
        nc.vector.memset(ones_col[:], 1.0)

        def load_w(dten, nm):  # [256,256] -> two [128,256] tiles in MM dtype
            tiles = []
            for k in range(2):
                wf = sbw.tile([P, D], F32, tag="wload")
                nc.sync.dma_start(out=wf[:], in_=dten[k * P:(k + 1) * P, :])
                wm = const.tile([P, D], MMDT, tag=f"w_{nm}_{k}")
                nc.vector.tensor_copy(out=wm[:], in_=wf[:])
                tiles.append(wm)
            return tiles

        W0 = load_w(W0_d, "w0"); W1 = load_w(W1_d, "w1")
        We = load_w(We_d, "we"); Wd = load_w(Wd_d, "wd")

        def bcast(dten, nm):  # [1,256] -> [128,256]
            row = sbw.tile([1, D], F32, tag="brow")
            nc.sync.dma_start(out=row[:], in_=dten[:])
            out = const.tile([P, D], F32, tag=f"bc_{nm}")
            nc.gpsimd.partition_broadcast(out[:], row[:])
            return out

        al0 = bcast(al0_d, "al0"); ar0 = bcast(ar0_d, "ar0")
        al1 = bcast(al1_d, "al1"); ar1 = bcast(ar1_d, "ar1")
        ald = bcast(ald_d, "ald"); ard = bcast(ard_d, "ard")
        mtok = bcast(mtok_d, "mtok")

        # metadata residents
        gsrc = meta.tile([P, T], I32)
        nc.sync.dma_start(out=gsrc[:], in_=gsrc_d[:])
        gseg = meta.tile([P, T], F32)
        nc.sync.dma_start(out=gseg[:], in_=gseg_d[:])
        pi = meta.tile([P, NBLK], I32)
        nc.sync.dma_start(out=pi[:], in_=pi_d[:])
        mcols = {}
        for nm, dt_ in [("tokflag", tokflag_d), ("tokkeep", tokkeep_d),
                        ("maskkeep", maskkeep_d), ("lossflag", lossflag_d)]:
            mt = meta.tile([P, NBLK], F32, tag=f"m_{nm}")
            nc.sync.dma_start(out=mt[:], in_=dt_[:])
            mcols[nm] = mt

        # layer residents: er per block
        er1 = res.tile([P, NBLK * H], F32)
        er2 = res.tile([P, NBLK * H], F32)
        erd = res.tile([P, NBLK], F32)
        loss_acc = res.tile([P, 1], F32)
        nc.vector.memset(loss_acc[:], 0.0)

        # DRAM tables / bounces
        tabA = dram.tile([TROWS, ROW_E], F32)
        tabB = dram.tile([TROWS, ROW_E], F32)
        tabC = dram.tile([TROWS, ROW_D], F32)
        bnA = dram.tile([NL, ROW_E], F32)
        bnB = dram.tile([NL, ROW_E], F32)
        bnC = dram.tile([NL, ROW_D], F32)

        padrow_e = const.tile([1, ROW_E], F32)
        nc.vector.memset(padrow_e[:], 0.0)
        nc.vector.memset(padrow_e[:, D:], -1e30)
        padrow_d = const.tile([1, ROW_D], F32)
        nc.vector.memset(padrow_d[:], 0.0)
        nc.vector.memset(padrow_d[:, D:], -1e30)

        # -------------- helpers -------------------------------------
        def transpose_128(src_ap, out_dt=MMDT):
            """[128,128] f32 slice -> SBUF tile in out_dt (via PE transpose)."""
            pst = ps_big.tile([P, P], F32, tag="ptr")
            nc.tensor.transpose(out=pst[:], in_=src_ap, identity=ident[:])
            s = sbw.tile([P, P], out_dt, tag="trc")
            nc.vector.tensor_copy(out=s[:], in_=pst[:])
            return s

        def dense(x_sb, Wt, nrow=P):
            """x[128,256] @ W(2 tiles) -> PSUM [128,256] (f32)."""
            xT = [transpose_128(x_sb[:, k * P:(k + 1) * P]) for k in range(2)]
            ph = ps_big.tile([P, D], F32, tag="pdense")
            for k in range(2):
                nc.tensor.matmul(out=ph[:], lhsT=xT[k][:], rhs=Wt[k][:],
                                 start=(k == 0), stop=(k == 1))
            return ph

        def head_reduce(h_ps, a_bcast, out_ap, nh):
            """out[p, nh] = sum over dh of h[p, nh, dh] * a[p, nh, dh]."""
            tmp = sbw.tile([P, D], F32, tag="hr")
            nc.vector.tensor_tensor(out=tmp[:], in0=h_ps[:], in1=a_bcast[:],
                                    op=OP.mult)
            nc.vector.tensor_reduce(
                out=out_ap, in_=tmp[:].rearrange('p (h d) -> p h d', h=nh),
                axis=mybir.AxisListType.X, op=OP.add)

        # ================= stage A: masking + dense L1 ================
        for b in range(NBLK):
            nrow = P if b < NBLK - 1 else LASTB
            xgt = sbw.tile([P, D], F32, tag="xg")
            nc.gpsimd.indirect_dma_start(
                out=xgt[:], out_offset=None, in_=xg_d[:],
                in_offset=bass.IndirectOffsetOnAxis(ap=pi[:, b:b + 1], axis=0))
            ux = sbw.tile([P, D], F32, tag="ux")
            # ux = xg*tokkeep + mtok*tokflag
            nc.vector.tensor_scalar(out=ux[:], in0=xgt[:],
                                    scalar1=mcols["tokkeep"][:, b:b + 1],
                                    scalar2=None, op0=OP.mult)
            ux2 = sbw.tile([P, D], F32, tag="ux2")
            nc.vector.scalar_tensor_tensor(
                out=ux2[:], in0=mtok[:], scalar=mcols["tokflag"][:, b:b + 1],
                in1=ux[:], op0=OP.mult, op1=OP.add)
            hp = dense(ux2, W0)
            rowA = sbw.tile([P, ROW_E], F32, tag="rowA")
            nc.vector.tensor_copy(out=rowA[:, :D], in_=hp[:])
            head_reduce(hp, al0, rowA[:, D:], H)
            head_reduce(hp, ar0, er1[:, b * H:(b + 1) * H], H)
            nc.sync.dma_start(out=bnA[b * P:b * P + nrow, :], in_=rowA[:nrow, :])

        nc.gpsimd.collective_compute(
            "AllGather", OP.bypass, replica_groups=[list(range(NCORES))],
            ins=[bnA[:].opt()], outs=[tabA[:N, :].opt()])
        nc.sync.dma_start(out=tabA[N:N + 1, :], in_=padrow_e[:])

        # ================= edge phase =================================
        def edge_layer(tab, row_w, nh, er_res, epilogue):
            """Run GAT edge phase over all blocks; epilogue(b, nrow, ps) consumes
            the accumulated PSUM [128, row_w] (num | den)."""
            col = 0
            for b in range(NBLK):
                nrow = P if b < NBLK - 1 else LASTB
                tb = tiles_b[b]
                acc = ps_acc.tile([P, row_w], F32, tag="acc")
                for t in range(tb):
                    tc_ = col + t
                    g = sbg.tile([P, row_w], F32, tag="g")
                    nc.gpsimd.indirect_dma_start(
                        out=g[:], out_offset=None, in_=tab[:],
                        in_offset=bass.IndirectOffsetOnAxis(
                            ap=gsrc[:, tc_:tc_ + 1], axis=0))
                    s0 = sbw.tile([P, P], MMDT, tag="s0")
                    nc.vector.tensor_scalar(out=s0[:], in0=iota[:],
                                            scalar1=gseg[:, tc_:tc_ + 1],
                                            scalar2=None, op0=OP.is_equal)
                    sT = transpose_128(s0[:].bitcast(F32), out_dt=F32)
                    pse = ps_er.tile([P, nh], F32, tag="pse")
                    nc.tensor.matmul(out=pse[:], lhsT=sT[:],
                                     rhs=er_res[:, b * nh:(b + 1) * nh],
                                     start=True, stop=True)
                    s1 = sbs.tile([P, nh], F32, tag="s1")
                    nc.vector.tensor_tensor(out=s1[:], in0=g[:, D:D + nh],
                                            in1=pse[:], op=OP.add)
                    s2 = sbs.tile([P, nh], F32, tag="s2")
                    nc.vector.scalar_tensor_tensor(out=s2[:], in0=s1[:],
                                                   scalar=NEG_SLOPE, in1=s1[:],
                                                   op0=OP.mult, op1=OP.max)
                    p = sbs.tile([P, nh], F32, tag="p")
                    nc.scalar.activation(p[:], s2[:], AF.Exp)
                    ph = sbg.tile([P, row_w], MMDT, tag="ph")
                    if nh > 1:
                        nc.vector.tensor_tensor(
                            out=ph[:, :D].rearrange('p (h d) -> p h d', h=nh),
                            in0=g[:, :D].rearrange('p (h d) -> p h d', h=nh),
                            in1=p[:].to_broadcast([P, nh, DH]), op=OP.mult)
                    else:
                        nc.vector.tensor_scalar(out=ph[:, :D], in0=g[:, :D],
                                                scalar1=p[:, :1], scalar2=None,
                                                op0=OP.mult)
                    if row_w - D == nh:
                        nc.gpsimd.tensor_copy(out=ph[:, D:], in_=p[:])
                    else:
                        nc.gpsimd.tensor_copy(
                            out=ph[:, D:], in_=p[:].to_broadcast([P, row_w - D]))
                    nc.tensor.matmul(out=acc[:], lhsT=s0[:], rhs=ph[:],
                                     start=(t == 0), stop=(t == tb - 1))
                epilogue(b, nrow, acc)
                col += tb

        def normalize(acc, nh):
            """alpha-normalize accumulated PSUM -> SBUF [128, 256]."""
            den = sbs.tile([P, nh], F32, tag="den")
            nc.vector.tensor_scalar(out=den[:], in0=acc[:, D:D + nh], scalar1=1e-9,
                                    scalar2=None, op0=OP.add)
            rden = sbs.tile([P, nh], F32, tag="rden")
            nc.vector.reciprocal(out=rden[:], in_=den[:])
            out = sbw.tile([P, D], F32, tag="attn")
            if nh > 1:
                nc.vector.tensor_tensor(
                    out=out[:].rearrange('p (h d) -> p h d', h=nh),
                    in0=acc[:, :D].rearrange('p (h d) -> p h d', h=nh),
                    in1=rden[:].to_broadcast([P, nh, DH]), op=OP.mult)
            else:
                nc.vector.tensor_scalar(out=out[:], in0=acc[:, :D],
                                        scalar1=rden[:, :1], scalar2=None,
                                        op0=OP.mult)
            return out

        def elu(x_sb):
            m = sbw.tile([P, D], F32, tag="elu_m")
            nc.vector.tensor_scalar_min(out=m[:], in0=x_sb[:], scalar1=0.0)
            e = sbw.tile([P, D], F32, tag="elu_e")
            nc.scalar.activation(e[:], m[:], AF.Exp)
            r = sbw.tile([P, D], F32, tag="elu_r")
            nc.vector.tensor_scalar_max(out=r[:], in0=x_sb[:], scalar1=0.0)
            o = sbw.tile([P, D], F32, tag="elu_o")
            nc.vector.scalar_tensor_tensor(out=o[:], in0=e[:], scalar=-1.0,
                                           in1=r[:], op0=OP.add, op1=OP.add)
            return o

        # ---- layer 1 epilogue: act1 -> h2/el2/er2 -> bounceB ----
        def ep1(b, nrow, acc):
            act = elu(normalize(acc, H))
            hp = dense(act, W1)
            rowB = sbw.tile([P, ROW_E], F32, tag="rowB")
            nc.vector.tensor_copy(out=rowB[:, :D], in_=hp[:])
            head_reduce(hp, al1, rowB[:, D:], H)
            head_reduce(hp, ar1, er2[:, b * H:(b + 1) * H], H)
            nc.sync.dma_start(out=bnB[b * P:b * P + nrow, :], in_=rowB[:nrow, :])

        # ---- layer 2 epilogue: enc_rep out; rep -> tabC rows ----
        def ep2(b, nrow, acc):
            act = elu(normalize(acc, H))
            nc.sync.dma_start(out=out_enc[b * P:b * P + nrow, :], in_=act[:nrow, :])
            repp = dense(act, We)
            rep = sbw.tile([P, D], F32, tag="rep")
            nc.vector.tensor_scalar(out=rep[:], in0=repp[:],
                                    scalar1=mcols["maskkeep"][:, b:b + 1],
                                    scalar2=None, op0=OP.mult)
            hp = dense(rep, Wd)
            rowC = sbw.tile([P, ROW_D], F32, tag="rowC")
            nc.vector.tensor_copy(out=rowC[:, :D], in_=hp[:])
            nc.vector.memset(rowC[:, D:], 0.0)
            head_reduce(hp, ald, rowC[:, D:D + 1], 1)
            head_reduce(hp, ard, erd[:, b:b + 1], 1)
            nc.sync.dma_start(out=bnC[b * P:b * P + nrow, :], in_=rowC[:nrow, :])

        # ---- decoder epilogue: recon -> loss ----
        def epd(b, nrow, acc):
            recon = normalize(acc, 1)
            xb = sbw.tile([P, D], F32, tag="lx")
            nc.sync.dma_start(out=xb[:], in_=xg_d[b * P:(b + 1) * P, :])
            junk = sbw.tile([P, D], F32, tag="ljunk")
            dot = sbs.tile([P, 1], F32, tag="ldot")
            nc.vector.scalar_tensor_tensor(out=junk[:], in0=recon[:], scalar=1.0,
                                           in1=xb[:], op0=OP.mult, op1=OP.mult,
                                           accum_out=dot[:])
            nx = sbs.tile([P, 1], F32, tag="lnx")
            nc.scalar.activation(junk[:], xb[:], AF.Square, accum_out=nx[:])
            ny = sbs.tile([P, 1], F32, tag="lny")
            nc.scalar.activation(junk[:], recon[:], AF.Square, accum_out=ny[:])
            sx = sbs.tile([P, 1], F32, tag="lsx")
            nc.scalar.activation(sx[:], nx[:], AF.Sqrt)
            sy = sbs.tile([P, 1], F32, tag="lsy")
            nc.scalar.activation(sy[:], ny[:], AF.Sqrt)
            nc.vector.tensor_scalar_add(out=sx[:], in0=sx[:], scalar1=1e-12)
            nc.vector.tensor_scalar_add(out=sy[:], in0=sy[:], scalar1=1e-12)
            dn = sbs.tile([P, 1], F32, tag="ldn")
            nc.vector.tensor_tensor(out=dn[:], in0=sx[:], in1=sy[:], op=OP.mult)
            rcp = sbs.tile([P, 1], F32, tag="lrcp")
            nc.vector.reciprocal(out=rcp[:], in_=dn[:])
            cosv = sbs.tile([P, 1], F32, tag="lcos")
            nc.vector.tensor_tensor(out=cosv[:], in0=dot[:], in1=rcp[:], op=OP.mult)
            u = sbs.tile([P, 1], F32, tag="lu")
            nc.vector.tensor_scalar(out=u[:], in0=cosv[:], scalar1=-1.0,
                                    scalar2=1.0, op0=OP.mult, op1=OP.add)
            u2 = sbs.tile([P, 1], F32, tag="lu2")
            nc.vector.tensor_tensor(out=u2[:], in0=u[:], in1=u[:], op=OP.mult)
            nc.vector.tensor_scalar(out=u2[:], in0=u2[:],
                                    scalar1=mcols["lossflag"][:, b:b + 1],
                                    scalar2=None, op0=OP.mult)
            nc.vector.tensor_tensor(out=loss_acc[:], in0=loss_acc[:], in1=u2[:],
                                    op=OP.add)

        # ---- run layers ----
        edge_layer(tabA, ROW_E, H, er1, ep1)
        if N_EDGE_LAYERS >= 2:
            nc.gpsimd.collective_compute(
                "AllGather", OP.bypass, replica_groups=[list(range(NCORES))],
                ins=[bnB[:].opt()], outs=[tabB[:N, :].opt()])
            nc.sync.dma_start(out=tabB[N:N + 1, :], in_=padrow_e[:])
            edge_layer(tabB, ROW_E, H, er2, ep2)
        if N_EDGE_LAYERS >= 3:
            nc.gpsimd.collective_compute(
                "AllGather", OP.bypass, replica_groups=[list(range(NCORES))],
                ins=[bnC[:].opt()], outs=[tabC[:N, :].opt()])
            nc.sync.dma_start(out=tabC[N:N + 1, :], in_=padrow_d[:])
            edge_layer(tabC, ROW_D, 1, erd, epd)

        # loss partial: partition-sum via matmul with ones
        psl = ps_er.tile([1, 1], F32, tag="pse")
        nc.tensor.matmul(out=psl[:], lhsT=loss_acc[:], rhs=ones_col[:],
                         start=True, stop=True)
        lsb = sbs.tile([1, 1], F32, tag="lsb")
        nc.vector.tensor_copy(out=lsb[:], in_=psl[:])
        nc.sync.dma_start(out=out_loss[:], in_=lsb[:])

    nc.compile()
    return nc


# ======================= entry point ====================================

_CACHE = {}


def kernel(**inputs):
    per_core, tiles_b, NXG = _shard(inputs)
    key = (tuple(tiles_b), NXG)
    if key not in _CACHE:
        _CACHE[key] = _build(tiles_b, NXG)
    nc = _CACHE[key]

    wkeys = {
        "W0": np.asarray(inputs["W_enc0"], np.float32),
        "W1": np.asarray(inputs["W_enc1"], np.float32),
        "We2d": np.asarray(inputs["W_e2d"], np.float32),
        "Wd": np.asarray(inputs["W_dec"], np.float32),
        "al0": np.asarray(inputs["al0"], np.float32).reshape(1, D),
        "ar0": np.asarray(inputs["ar0"], np.float32).reshape(1, D),
        "al1": np.asarray(inputs["al1"], np.float32).reshape(1, D),
        "ar1": np.asarray(inputs["ar1"], np.float32).reshape(1, D),
        "ald": np.asarray(inputs["ald"], np.float32).reshape(1, D),
        "ard": np.asarray(inputs["ard"], np.float32).reshape(1, D),
        "mtok": np.asarray(inputs["mask_token"], np.float32).reshape(1, D),
    }
    in_maps = []
    for c in range(NCORES):
        m = dict(per_core[c])
        m.update(wkeys)
        in_maps.append(m)

    r = run_bass_kernel_spmd(nc, in_maps, list(range(NCORES)))
    enc_rep = np.concatenate([r.results[c]["enc_rep"] for c in range(NCORES)],
                             axis=0)
    loss = np.float32(sum(float(r.results[c]["loss_part"][0, 0])
                          for c in range(NCORES)) / NUM_MASK)
    return loss, enc_rep


def _make_runner(nc, in_maps):
    """Build a reusable jitted SPMD runner with device-resident inputs.
    Returns run() -> per-call wall seconds (blocking)."""
    import jax, time
    import jax.numpy as jnp
    from functools import partial
    from jax.sharding import Mesh, PartitionSpec, NamedSharding
    from jax.experimental.shard_map import shard_map
    from concourse import bass2jax, mybir as mb
    bass2jax.install_neuronx_cc_hook()

    partition_name = nc.partition_id_tensor.name if nc.partition_id_tensor else None
    in_names, out_names, out_avals, zero_shapes = [], [], [], []
    for alloc in nc.m.functions[0].allocations:
        if not isinstance(alloc, mb.MemoryLocationSet):
            continue
        name = alloc.memorylocations[0].name
        if alloc.kind == "ExternalInput":
            if name != partition_name:
                in_names.append(name)
        elif alloc.kind == "ExternalOutput":
            out_names.append(name)
            shape = tuple(alloc.tensor_shape)
            dtype = mb.dt.np(alloc.dtype)
            out_avals.append(jax.core.ShapedArray(shape, dtype))
            zero_shapes.append((shape, dtype))
    n_params = len(in_names)
    all_names = list(in_names) + list(out_names)
    if partition_name is not None:
        all_names.append(partition_name)
    donate = tuple(range(n_params, n_params + len(out_names)))

    def _body(*args):
        operands = list(args)
        if partition_name is not None:
            operands.append(bass2jax.partition_id_tensor())
        return tuple(bass2jax._bass_exec_p.bind(
            *operands, out_avals=tuple(out_avals), in_names=tuple(all_names),
            out_names=tuple(out_names), lowering_input_output_aliases=(),
            sim_require_finite=True, sim_require_nnan=True, nc=nc))

    devices = jax.devices()[:NCORES]
    mesh = Mesh(np.asarray(devices), ("core",))
    pspec = PartitionSpec("core")
    sharded = jax.jit(
        shard_map(_body, mesh=mesh, in_specs=(pspec,) * (n_params + len(out_names)),
                  out_specs=(pspec,) * len(out_names), check_rep=False),
        donate_argnums=donate, keep_unused=True)

    sh = NamedSharding(mesh, pspec)
    dev_in = [jax.device_put(
        np.concatenate([np.asarray(in_maps[c][nm]) for c in range(NCORES)], axis=0), sh)
        for nm in in_names]
    zero_fns = [jax.jit(partial(jnp.zeros, (NCORES * shp[0],) + shp[1:], dt),
                        out_shardings=sh) for shp, dt in zero_shapes]

    def run():
        zouts = [f() for f in zero_fns]
        for z in zouts:
            z.block_until_ready()
        t0 = time.perf_counter()
        outs = sharded(*dev_in, *zouts)
        for o in outs:
            o.block_until_ready()
        return time.perf_counter() - t0

    return run


def time_exec(reps=10, **inputs):
    """Median per-run wall time minus trivial-kernel baseline (dispatch)."""
    per_core, tiles_b, NXG = _shard(inputs)
    key = (tuple(tiles_b), NXG)
    if key not in _CACHE:
        _CACHE[key] = _build(tiles_b, NXG)
    nc = _CACHE[key]
    wkeys = {
        "W0": np.asarray(inputs["W_enc0"], np.float32),
        "W1": np.asarray(inputs["W_enc1"], np.float32),
        "We2d": np.asarray(inputs["W_e2d"], np.float32),
        "Wd": np.asarray(inputs["W_dec"], np.float32),
        "al0": np.asarray(inputs["al0"], np.float32).reshape(1, D),
        "ar0": np.asarray(inputs["ar0"], np.float32).reshape(1, D),
        "al1": np.asarray(inputs["al1"], np.float32).reshape(1, D),
        "ar1": np.asarray(inputs["ar1"], np.float32).reshape(1, D),
        "ald": np.asarray(inputs["ald"], np.float32).reshape(1, D),
        "ard": np.asarray(inputs["ard"], np.float32).reshape(1, D),
        "mtok": np.asarray(inputs["mask_token"], np.float32).reshape(1, D),
    }
    in_maps = []
    for c in range(NCORES):
        m = dict(per_core[c]); m.update(wkeys)
        in_maps.append(m)

    run_full = _make_runner(nc, in_maps)
    run_base = _make_runner(_baseline_nc(),
                            [{"z": np.zeros((1, 1), np.float32)}] * NCORES)
    full = sorted(run_full() for _ in range(reps))
    base = sorted(run_base() for _ in range(reps))
    print(f"  runs full: {[f'{t*1e3:.2f}ms' for t in full]}")
    print(f"  runs base: {[f'{t*1e3:.2f}ms' for t in base]}")
    return max(full[len(full) // 2] - base[len(base) // 2], 0.0) * 1e9


_BASE = {}


def _baseline_nc():
    if "nc" not in _BASE:
        nc = bacc.Bacc("TRN2", debug=False, num_devices=NCORES)
        z_d = nc.dram_tensor("z", [1, 1], F32, kind="ExternalInput").ap()
        o_d = nc.dram_tensor("o", [1, 1], F32, kind="ExternalOutput").ap()
        with tile.TileContext(nc) as tc, ExitStack() as ctx:
            sb = ctx.enter_context(tc.tile_pool(name="sb", bufs=1))
            t = sb.tile([1, 1], F32)
            nc.sync.dma_start(out=t[:], in_=z_d[:])
            nc.sync.dma_start(out=o_d[:], in_=t[:])
        nc.compile()
        _BASE["nc"] = nc
    return _BASE["nc"]


if __name__ == "__main__":
    import time
    import reference as R
    ins = {k: np.asarray(v) for k, v in R.setup_inputs().items()}
    t0 = time.time()
    loss, enc = kernel(**ins)
    print(f"kernel: {time.time()-t0:.1f}s loss={loss}")
